# revision 1
# baseline (speedup 1.0000x reference)
"""Causal depthwise conv1d (K=4) + SiLU, sharded over 8 NeuronCores.

Full shapes: x [4, 8192, 2048] f32, weight [2048, 4] f32 -> y [4, 8192, 2048] f32.

Strategy: tensor-parallel over the hidden/channel dim (fully channel
independent, no halo exchange). Each core gets 256 channels, reorganized
host-side to channel-major [B*256, 3+S] (3 leading zero columns provide the
causal padding) so the conv runs along the free dim with channels on SBUF
partitions.

Compute: all 4 taps run on the TensorEngine as float32r diagonal-matrix
matmuls accumulating in PSUM (psum[c,t] += w_i[c] * x[c, t-3+i] via
diag(w_i) @ x_shifted). The diagonal weight matrices are built on-device
(gpsimd affine_select identity mask x per-partition tap scalar on DVE). DVE
rounds each input tile fp32 -> fp32r (the fp32r matmul contract requires
rounded producers); ACT applies SiLU straight out of PSUM and triggers the
output DMA on its own HWDGE ring, with inputs streaming on SP's ring.

Raw bass (no Tile framework): the installed walrus codegen only accepts one
sync wait per compute instruction, so all synchronization is explicit wait_ge
sequencer instructions. Per-buffer-slot DMA semaphores keep concurrent DMA
completion increments unambiguous. Sem increments fire at instruction
completion, but the sequencer runs ahead, so consumers of an engine's result
always gate on that completion increment (including same-engine self-waits
before DMA triggers).
"""

import contextlib

import numpy as np

B, S, H, K = 4, 8192, 2048, 4
N_CORES = 8
HC = H // N_CORES          # 256 channels per core
ROWS = B * HC              # 1024 rows per core, row r = b*HC + c
NU = ROWS // 128           # 8 partition units
T = 2048                   # token tile
NT = S // T
NTILES = NU * NT           # 32
NB = 6                     # buffers per tile kind
NC_CHUNK = 512             # one PSUM bank of fp32
NCHUNKS = T // NC_CHUNK

_last_results = None       # test harness introspection (exec_time_ns etc.)
_ACT_FUNC = "Silu"         # sim override hook (CoreSim lacks Silu)


def _build_program():
    from concourse import bass, mybir

    f32 = mybir.dt.float32
    f32r = mybir.dt.float32r
    AF = mybir.ActivationFunctionType

    nc = bass.Bass()
    # x arrives with 3 leading zero columns (causal padding): [ROWS, 3+S]
    x_d = nc.declare_dram_parameter("x", [ROWS, S + 3], f32, isOutput=False)
    w_d = nc.declare_dram_parameter("w", [128, NU * K + 1], f32, isOutput=False)
    y_d = nc.declare_dram_parameter("y", [ROWS, S], f32, isOutput=True)

    with contextlib.ExitStack() as st:
        wt = st.enter_context(nc.sbuf_tensor("wt", [128, NU * K + 1], f32))
        eye = st.enter_context(nc.sbuf_tensor("eye", [128, 128], f32))
        wtr = st.enter_context(nc.sbuf_tensor("wtr", [128, NU * K * 128], f32r))
        xts = [
            st.enter_context(nc.sbuf_tensor(f"xt{i}", [128, T + 3], f32))
            for i in range(NB)
        ]
        xrs = [
            st.enter_context(nc.sbuf_tensor(f"xr{i}", [128, T + 3], f32r))
            for i in range(NB)
        ]
        yts = [
            st.enter_context(nc.sbuf_tensor(f"yt{i}", [128, T], f32))
            for i in range(NB)
        ]
        pss = [
            st.enter_context(nc.psum_tensor(f"ps{i}", [128, T], f32))
            for i in range(2)
        ]
        zb = wt[:, NU * K : NU * K + 1]           # zeros column (Silu bias)

        def wdiag(k, i):
            u = k // NT
            c0 = (u * K + i) * 128
            return wtr[:, c0 : c0 + 128]

        def x_rows(k):
            r0 = (k // NT) * 128
            return r0, r0 + 128

        with (
            nc.Block() as block,
            nc.semaphore("wsem") as wsem,
            nc.semaphore("esem") as esem,
            nc.semaphore("act") as act,
            nc.semaphore("dve") as dve,
            nc.semaphore("pe") as pe,
            contextlib.ExitStack() as sems,
        ):
            din = [
                sems.enter_context(nc.semaphore(f"din{i}")) for i in range(NB)
            ]
            dout = [
                sems.enter_context(nc.semaphore(f"dout{i}")) for i in range(NB)
            ]

            @block.sync
            def _(sync):
                sync.dma_start(out=wt[:, :], in_=w_d[:, :]).then_inc(wsem, 16)
                for k in range(NTILES):
                    r0, r1 = x_rows(k)
                    t0 = (k % NT) * T
                    if k >= NB:
                        # xt slot free once DVE rounded tile k-NB out of it
                        prev = k - NB
                        sync.wait_ge(dve, 2 if prev == 0 else prev + 3)
                    # padded coords: window [t0-3, t0+T) = x_d cols [t0, t0+T+3)
                    sync.dma_start(
                        out=xts[k % NB][:, :],
                        in_=x_d[r0:r1, t0 : t0 + T + 3],
                    ).then_inc(din[k % NB], 16)

            @block.gpsimd
            def _(gpsimd):
                # identity mask for the diagonal weight build (affine_select
                # only exists on gpsimd)
                gpsimd.memset(eye[:, :], 1.0)
                gpsimd.affine_select(
                    out=eye[:, :], in_=eye[:, :],
                    pattern=[[1, 128]], base=0, channel_multiplier=-1,
                    compare_op=mybir.AluOpType.is_equal, fill=0.0,
                ).then_inc(esem)

            @block.vector
            def _(vector):
                # build the diagonal weight matrices on-device; unit 0 first
                # so PE can start, the rest behind tile 0's rounding.
                # dve incs: #1 unit-0 diags, #2 round_0, #3 remaining diags,
                # #k+3 round_k (k>=1)
                def diag_build(units):
                    for u in units:
                        for i in range(K):
                            mul = vector.tensor_scalar_mul(
                                wtr[:, (u * K + i) * 128 : (u * K + i + 1) * 128],
                                eye[:, :],
                                wt[:, u * K + i : u * K + i + 1],
                            )
                    return mul

                def round_tile(k):
                    vector.wait_ge(din[k % NB], 16 * (k // NB + 1))
                    if k >= NB:
                        # xr slot free once PE consumed tile k-NB
                        vector.wait_ge(pe, k - NB + 1)
                    return vector.tensor_copy(
                        out=xrs[k % NB][:, :], in_=xts[k % NB][:, :]
                    )

                vector.wait_ge(wsem, 16)
                vector.wait_ge(esem, 1)
                diag_build([0]).then_inc(dve)
                round_tile(0).then_inc(dve)
                diag_build(range(1, NU)).then_inc(dve)
                for k in range(1, NTILES):
                    round_tile(k).then_inc(dve)

            @block.tensor
            def _(tensor):
                for k in range(NTILES):
                    # k=0: unit-0 diags + round_0; k>=1: all diags + round_k
                    tensor.wait_ge(dve, 2 if k == 0 else k + 3)
                    if k >= 2:
                        # psum buffer free once silu of tile k-2 done
                        tensor.wait_ge(act, k - 1)
                    ps = pss[k % 2]
                    xr = xrs[k % NB]
                    for c in range(NCHUNKS):
                        c0 = c * NC_CHUNK
                        for i in range(K):
                            mm = tensor.matmul(
                                ps[:, c0 : c0 + NC_CHUNK],
                                wdiag(k, i),
                                xr[:, c0 + i : c0 + i + NC_CHUNK],
                                start=(i == 0),
                                stop=(i == K - 1),
                                skip_group_check=True,
                            )
                    mm.then_inc(pe)

            @block.scalar
            def _(scalar):
                func = getattr(AF, _ACT_FUNC)
                for k in range(NTILES):
                    scalar.wait_ge(pe, k + 1)
                    if k >= NB:
                        # yt slot's previous store (tile k-NB) must be done
                        scalar.wait_ge(dout[k % NB], 16 * (k // NB))
                    scalar.activation(
                        out=yts[k % NB][:, :], in_=pss[k % 2][:, :],
                        func=func,
                        bias=0.0 if func == AF.Copy else zb,
                        scale=1.0,
                    ).then_inc(act)
                    # the DMA trigger races ahead of the still-streaming
                    # activation write; self-wait on its completion inc
                    scalar.wait_ge(act, k + 1)
                    r0, r1 = x_rows(k)
                    t0 = (k % NT) * T
                    scalar.dma_start(
                        out=y_d[r0:r1, t0 : t0 + T], in_=yts[k % NB][:, :]
                    ).then_inc(dout[k % NB], 16)
                for i in range(NB):
                    n_stores = len([k for k in range(NTILES) if k % NB == i])
                    scalar.wait_ge(dout[i], 16 * n_stores)

    return nc


def kernel(x, weight):
    global _last_results
    from concourse.bass_utils import run_bass_kernel_spmd

    x = np.asarray(x, dtype=np.float32)
    weight = np.asarray(weight, dtype=np.float32)

    nc = _build_program()

    in_maps = []
    for core in range(N_CORES):
        sl = slice(core * HC, (core + 1) * HC)
        # [B, S, HC] -> [B, HC, S] -> [ROWS, S] with 3 leading zero columns
        # (the causal padding), row r = b*HC + c
        xs = np.zeros((ROWS, S + 3), np.float32)
        xs[:, 3:] = x[:, :, sl].transpose(0, 2, 1).reshape(ROWS, S)
        ws = weight[sl, :]  # (HC, K)
        w_host = np.zeros((128, NU * K + 1), np.float32)
        for u in range(NU):
            blk = u % (HC // 128)
            w_host[:, u * K : (u + 1) * K] = ws[blk * 128 : (blk + 1) * 128, :]
        in_maps.append({"x": xs, "w": w_host})

    res = run_bass_kernel_spmd(nc, in_maps, list(range(N_CORES)))
    _last_results = res

    out = np.empty((B, S, H), np.float32)
    for core in range(N_CORES):
        sl = slice(core * HC, (core + 1) * HC)
        yc = res.results[core]["y"].reshape(B, HC, S)
        out[:, :, sl] = yc.transpose(0, 2, 1)
    return out



# revision 2
# speedup vs baseline: 1.6065x; 1.6065x over previous
"""Causal depthwise conv1d (K=4) + SiLU, sharded over 8 NeuronCores.

Full shapes: x [4, 8192, 2048] f32, weight [2048, 4] f32 -> y [4, 8192, 2048] f32.

Strategy: tensor-parallel over the hidden/channel dim (fully channel
independent, no halo exchange). Each core gets 256 channels, reorganized
host-side to channel-major [B*256, 3+S] (3 leading zero columns provide the
causal padding) so the conv runs along the free dim with channels on SBUF
partitions.

The kernel is HBM-bandwidth bound, so all HBM traffic is bf16: x is converted
host-side (RNE via ml_dtypes), y is written bf16 and upconverted host-side.
That halves traffic vs fp32 (rel-err budget 2e-2 >> bf16's ~2e-3).

Compute split so every engine stays under the ~2.9us/tile bf16 DMA budget:
taps 0-2 run on the TensorEngine as bf16 diagonal-matrix matmuls accumulating
in PSUM (the 32 [128,128] diag matrices are built host-side and DMA'd once);
tap 3 is folded by the DVE in one scalar_tensor_tensor
(z = x3*w3 + psum -> sbuf bf16); ACT applies SiLU (z -> bf16 out) and triggers
the output DMA on its own HWDGE ring, with inputs streaming on SP's ring.

Raw bass (no Tile framework): the installed walrus codegen only accepts one
sync wait per compute instruction, so all synchronization is explicit wait_ge
sequencer instructions. Per-buffer-slot DMA semaphores keep concurrent DMA
completion increments unambiguous. Sem increments fire at instruction
completion, but the sequencer runs ahead, so consumers of an engine's result
always gate on that completion increment (including same-engine self-waits
before DMA triggers).
"""

import contextlib

import numpy as np

B, S, H, K = 4, 8192, 2048, 4
N_CORES = 8
HC = H // N_CORES          # 256 channels per core
ROWS = B * HC              # 1024 rows per core, row r = b*HC + c
NU = ROWS // 128           # 8 partition units
T = 2048                   # token tile
NT = S // T
NTILES = NU * NT           # 32
NB = 6                     # buffers per tile kind
NZ = 3                     # z (pre-silu) buffers
NC_CHUNK = 512             # one PSUM bank of fp32
NCHUNKS = T // NC_CHUNK
PE_TAPS = 3                # taps on the TensorEngine; tap 3 folds in on DVE

_last_results = None       # test harness introspection (exec_time_ns etc.)
_ACT_FUNC = "Silu"         # sim override hook (CoreSim lacks Silu)


def _build_program():
    from concourse import bass, mybir

    f32 = mybir.dt.float32
    bf16 = mybir.dt.bfloat16
    AF = mybir.ActivationFunctionType
    ALU = mybir.AluOpType

    nc = bass.Bass()
    # x arrives with 3 leading zero columns (causal padding): [ROWS, 3+S]
    x_d = nc.declare_dram_parameter("x", [ROWS, S + 3], bf16, isOutput=False)
    # 32 host-built [128,128] diag(w) blocks, unit-major then tap
    wd_d = nc.declare_dram_parameter("wd", [128, NU * K * 128], bf16,
                                     isOutput=False)
    # raw weights (fp32): stt scalar columns + a zeros column (Silu bias)
    w_d = nc.declare_dram_parameter("w", [128, NU * K + 1], f32, isOutput=False)
    y_d = nc.declare_dram_parameter("y", [ROWS, S], bf16, isOutput=True)

    with contextlib.ExitStack() as st:
        wt = st.enter_context(nc.sbuf_tensor("wt", [128, NU * K + 1], f32))
        wdg = st.enter_context(nc.sbuf_tensor("wdg", [128, NU * K * 128], bf16))
        xts = [
            st.enter_context(nc.sbuf_tensor(f"xt{i}", [128, T + 3], bf16))
            for i in range(NB)
        ]
        zts = [
            st.enter_context(nc.sbuf_tensor(f"zt{i}", [128, T], bf16))
            for i in range(NZ)
        ]
        yts = [
            st.enter_context(nc.sbuf_tensor(f"yt{i}", [128, T], bf16))
            for i in range(NB)
        ]
        pss = [
            st.enter_context(nc.psum_tensor(f"ps{i}", [128, T], f32))
            for i in range(2)
        ]
        zb = wt[:, NU * K : NU * K + 1]           # zeros column (Silu bias)

        def wdiag(k, i):
            u = k // NT
            c0 = (u * K + i) * 128
            return wdg[:, c0 : c0 + 128]

        def w3col(k):
            u = k // NT
            return wt[:, u * K + 3 : u * K + 4]

        def x_rows(k):
            r0 = (k // NT) * 128
            return r0, r0 + 128

        with (
            nc.Block() as block,
            nc.semaphore("wsem") as wsem,
            nc.semaphore("dsem") as dsem,
            nc.semaphore("dvez") as dvez,
            nc.semaphore("act") as act,
            nc.semaphore("pe") as pe,
            contextlib.ExitStack() as sems,
        ):
            din = [
                sems.enter_context(nc.semaphore(f"din{i}")) for i in range(NB)
            ]
            dout = [
                sems.enter_context(nc.semaphore(f"dout{i}")) for i in range(NB)
            ]

            @block.sync
            def _(sync):
                sync.dma_start(out=wt[:, :], in_=w_d[:, :]).then_inc(wsem, 16)
                for k in range(NTILES):
                    r0, r1 = x_rows(k)
                    t0 = (k % NT) * T
                    if k >= NB:
                        # xt slot free once DVE folded tap 3 of tile k-NB
                        sync.wait_ge(dvez, k - NB + 1)
                    # padded coords: window [t0-3, t0+T) = x_d cols [t0, t0+T+3)
                    sync.dma_start(
                        out=xts[k % NB][:, :],
                        in_=x_d[r0:r1, t0 : t0 + T + 3],
                    ).then_inc(din[k % NB], 16)

            @block.tensor
            def _(tensor):
                tensor.wait_ge(dsem, 16)
                for k in range(NTILES):
                    tensor.wait_ge(din[k % NB], 16 * (k // NB + 1))
                    if k >= 2:
                        # psum buffer free once DVE consumed tile k-2
                        tensor.wait_ge(dvez, k - 1)
                    ps = pss[k % 2]
                    xt = xts[k % NB]
                    for i in range(PE_TAPS):
                        for c in range(NCHUNKS):
                            c0 = c * NC_CHUNK
                            mm = tensor.matmul(
                                ps[:, c0 : c0 + NC_CHUNK],
                                wdiag(k, i),
                                xt[:, c0 + i : c0 + i + NC_CHUNK],
                                start=(i == 0),
                                stop=(i == PE_TAPS - 1),
                                skip_group_check=True,
                            )
                    mm.then_inc(pe)

            @block.vector
            def _(vector):
                vector.wait_ge(wsem, 16)
                for k in range(NTILES):
                    vector.wait_ge(pe, k + 1)
                    if k >= NZ:
                        # z slot free once ACT read tile k-NZ
                        vector.wait_ge(act, k - NZ + 1)
                    # z = x3 * w3 + psum  (tap 3 fold)
                    vector.scalar_tensor_tensor(
                        out=zts[k % NZ][:, :],
                        in0=xts[k % NB][:, 3 : 3 + T],
                        scalar=w3col(k),
                        in1=pss[k % 2][:, :],
                        op0=ALU.mult,
                        op1=ALU.add,
                    ).then_inc(dvez)

            @block.scalar
            def _(scalar):
                scalar.dma_start(out=wdg[:, :], in_=wd_d[:, :]).then_inc(
                    dsem, 16
                )
                func = getattr(AF, _ACT_FUNC)
                for k in range(NTILES):
                    scalar.wait_ge(dvez, k + 1)
                    if k >= NB:
                        # yt slot's previous store (tile k-NB) must be done
                        scalar.wait_ge(dout[k % NB], 16 * (k // NB))
                    scalar.activation(
                        out=yts[k % NB][:, :], in_=zts[k % NZ][:, :],
                        func=func,
                        bias=0.0 if func == AF.Copy else zb,
                        scale=1.0,
                    ).then_inc(act)
                    # the DMA trigger races ahead of the still-streaming
                    # activation write; self-wait on its completion inc
                    scalar.wait_ge(act, k + 1)
                    r0, r1 = x_rows(k)
                    t0 = (k % NT) * T
                    scalar.dma_start(
                        out=y_d[r0:r1, t0 : t0 + T], in_=yts[k % NB][:, :]
                    ).then_inc(dout[k % NB], 16)
                for i in range(NB):
                    n_stores = len([k for k in range(NTILES) if k % NB == i])
                    scalar.wait_ge(dout[i], 16 * n_stores)

    return nc


def kernel(x, weight):
    global _last_results
    import ml_dtypes
    from concourse.bass_utils import run_bass_kernel_spmd

    bf16 = ml_dtypes.bfloat16
    x = np.asarray(x, dtype=np.float32)
    weight = np.asarray(weight, dtype=np.float32)

    nc = _build_program()

    in_maps = []
    for core in range(N_CORES):
        sl = slice(core * HC, (core + 1) * HC)
        # [B, S, HC] -> [B, HC, S] -> [ROWS, S] with 3 leading zero columns
        # (the causal padding), row r = b*HC + c
        xs = np.zeros((ROWS, S + 3), bf16)
        xs[:, 3:] = x[:, :, sl].transpose(0, 2, 1).reshape(ROWS, S).astype(bf16)
        ws = weight[sl, :]  # (HC, K)
        w_host = np.zeros((128, NU * K + 1), np.float32)
        wd_host = np.zeros((128, NU * K * 128), bf16)
        idx = np.arange(128)
        for u in range(NU):
            blk = u % (HC // 128)
            wu = ws[blk * 128 : (blk + 1) * 128, :]  # (128, K)
            w_host[:, u * K : (u + 1) * K] = wu
            for i in range(K):
                wd_host[idx, (u * K + i) * 128 + idx] = wu[:, i].astype(bf16)
        in_maps.append({"x": xs, "w": w_host, "wd": wd_host})

    res = run_bass_kernel_spmd(nc, in_maps, list(range(N_CORES)))
    _last_results = res

    out = np.empty((B, S, H), np.float32)
    for core in range(N_CORES):
        sl = slice(core * HC, (core + 1) * HC)
        yc = res.results[core]["y"].astype(np.float32).reshape(B, HC, S)
        out[:, :, sl] = yc.transpose(0, 2, 1)
    return out


# revision 3
# speedup vs baseline: 1.6283x; 1.0136x over previous
"""Causal depthwise conv1d (K=4) + SiLU, sharded over 8 NeuronCores.

Full shapes: x [4, 8192, 2048] f32, weight [2048, 4] f32 -> y [4, 8192, 2048] f32.

Strategy: tensor-parallel over the hidden/channel dim (fully channel
independent, no halo exchange). Each core gets 256 channels, reorganized
host-side to channel-major [B*256, 3+S] (3 leading zero columns provide the
causal padding) so the conv runs along the free dim with channels on SBUF
partitions.

The kernel is HBM-bandwidth bound, so all HBM traffic is bf16: x is converted
host-side (RNE via ml_dtypes), y is written bf16 and upconverted host-side.
That halves traffic vs fp32 (rel-err budget 2e-2 >> bf16's ~2e-3).

Compute split so every engine stays under the ~2.9us/tile bf16 DMA budget:
taps 0-2 run on the TensorEngine as bf16 diagonal-matrix matmuls accumulating
in PSUM (the 32 [128,128] diag matrices are built host-side and DMA'd once);
tap 3 is folded by the DVE scalar_tensor_tensor (z = x3*w3 + psum -> sbuf
bf16); ACT applies SiLU (z -> bf16 out) and triggers the output DMA on its
own HWDGE ring, with inputs streaming on SP's ring.

Ramp shaping (the steady state already runs at the HBM roofline): the diag
DMA is split so unit 0's block lands first and tile 0's input lands in two
halves, pulling the first matmul ~5us earlier; PE/DVE work half-tiles
(per-half sem incs) and the last tile's SiLU + store are split in half, so
the end-of-kernel dependency chain drains ~6us faster.

Raw bass (no Tile framework): the installed walrus codegen only accepts one
sync wait per compute instruction, so all synchronization is explicit wait_ge
sequencer instructions. Per-buffer-slot DMA semaphores keep concurrent DMA
completion increments unambiguous. Sem increments fire at instruction
completion, but the sequencer runs ahead, so consumers of an engine's result
always gate on that completion increment (including same-engine self-waits
before DMA triggers).
"""

import contextlib

import numpy as np

B, S, H, K = 4, 8192, 2048, 4
N_CORES = 8
HC = H // N_CORES          # 256 channels per core
ROWS = B * HC              # 1024 rows per core, row r = b*HC + c
NU = ROWS // 128           # 8 partition units
T = 2048                   # token tile
NT = S // T
NTILES = NU * NT           # 32
NB = 6                     # buffers per tile kind
NZ = 3                     # z (pre-silu) buffers
NC_CHUNK = 512             # one PSUM bank of fp32
NCHUNKS = T // NC_CHUNK
PE_TAPS = 3                # taps on the TensorEngine; tap 3 folds in on DVE
HT = T // 2                # half-tile (sem granularity for PE/DVE)
X0SPLIT = HT + 4           # tile-0 first-half DMA columns (covers chunks 0-1)

_last_results = None       # test harness introspection (exec_time_ns etc.)
_ACT_FUNC = "Silu"         # sim override hook (CoreSim lacks Silu)


def _build_program():
    from concourse import bass, mybir

    f32 = mybir.dt.float32
    bf16 = mybir.dt.bfloat16
    AF = mybir.ActivationFunctionType
    ALU = mybir.AluOpType

    nc = bass.Bass()
    # x arrives with 3 leading zero columns (causal padding): [ROWS, 3+S]
    x_d = nc.declare_dram_parameter("x", [ROWS, S + 3], bf16, isOutput=False)
    # 32 host-built [128,128] diag(w) blocks, unit-major then tap
    wd_d = nc.declare_dram_parameter("wd", [128, NU * K * 128], bf16,
                                     isOutput=False)
    # raw weights (fp32): stt scalar columns + a zeros column (Silu bias)
    w_d = nc.declare_dram_parameter("w", [128, NU * K + 1], f32, isOutput=False)
    y_d = nc.declare_dram_parameter("y", [ROWS, S], bf16, isOutput=True)

    with contextlib.ExitStack() as st:
        wt = st.enter_context(nc.sbuf_tensor("wt", [128, NU * K + 1], f32))
        wdg = st.enter_context(nc.sbuf_tensor("wdg", [128, NU * K * 128], bf16))
        xts = [
            st.enter_context(nc.sbuf_tensor(f"xt{i}", [128, T + 3], bf16))
            for i in range(NB)
        ]
        zts = [
            st.enter_context(nc.sbuf_tensor(f"zt{i}", [128, T], bf16))
            for i in range(NZ)
        ]
        yts = [
            st.enter_context(nc.sbuf_tensor(f"yt{i}", [128, T], bf16))
            for i in range(NB)
        ]
        pss = [
            st.enter_context(nc.psum_tensor(f"ps{i}", [128, T], f32))
            for i in range(2)
        ]
        zb = wt[:, NU * K : NU * K + 1]           # zeros column (Silu bias)

        def wdiag(k, i):
            u = k // NT
            c0 = (u * K + i) * 128
            return wdg[:, c0 : c0 + 128]

        def w3col(k):
            u = k // NT
            return wt[:, u * K + 3 : u * K + 4]

        def x_rows(k):
            r0 = (k // NT) * 128
            return r0, r0 + 128

        # cumulative din counts: tile 0 arrives as two half DMAs
        din_need = []          # din count PE needs before tile k (full tile)
        din_tot = [0] * NB
        for k in range(NTILES):
            din_tot[k % NB] += 32 if k == 0 else 16
            din_need.append(din_tot[k % NB])

        with (
            nc.Block() as block,
            nc.semaphore("wsem") as wsem,
            nc.semaphore("dsem0") as dsem0,
            nc.semaphore("dsem1") as dsem1,
            nc.semaphore("dvez") as dvez,
            nc.semaphore("act") as act,
            nc.semaphore("pe") as pe,
            contextlib.ExitStack() as sems,
        ):
            din = [
                sems.enter_context(nc.semaphore(f"din{i}")) for i in range(NB)
            ]
            dout = [
                sems.enter_context(nc.semaphore(f"dout{i}")) for i in range(NB)
            ]

            @block.sync
            def _(sync):
                for k in range(NTILES):
                    r0, r1 = x_rows(k)
                    t0 = (k % NT) * T
                    if k >= NB:
                        # xt slot free once DVE folded tap 3 of tile k-NB
                        sync.wait_ge(dvez, 2 * (k - NB) + 2)
                    # padded coords: window [t0-3, t0+T) = x_d cols [t0, t0+T+3)
                    if k == 0:
                        sync.dma_start(
                            out=xts[0][:, :X0SPLIT],
                            in_=x_d[r0:r1, :X0SPLIT],
                        ).then_inc(din[0], 16)
                        sync.dma_start(
                            out=xts[0][:, X0SPLIT : T + 3],
                            in_=x_d[r0:r1, X0SPLIT : T + 3],
                        ).then_inc(din[0], 16)
                    else:
                        sync.dma_start(
                            out=xts[k % NB][:, :],
                            in_=x_d[r0:r1, t0 : t0 + T + 3],
                        ).then_inc(din[k % NB], 16)

            @block.tensor
            def _(tensor):
                tensor.wait_ge(dsem0, 16)
                for k in range(NTILES):
                    if k == 4:
                        tensor.wait_ge(dsem1, 16)
                    if k >= 2:
                        # psum buffer free once DVE consumed tile k-2
                        tensor.wait_ge(dvez, 2 * (k - 2) + 2)
                    ps = pss[k % 2]
                    xt = xts[k % NB]
                    for h in range(2):
                        if k == 0:
                            tensor.wait_ge(din[0], 16 * (h + 1))
                        elif h == 0:
                            tensor.wait_ge(din[k % NB], din_need[k])
                        for c in range(2 * h, 2 * h + 2):
                            c0 = c * NC_CHUNK
                            for i in range(PE_TAPS):
                                mm = tensor.matmul(
                                    ps[:, c0 : c0 + NC_CHUNK],
                                    wdiag(k, i),
                                    xt[:, c0 + i : c0 + i + NC_CHUNK],
                                    start=(i == 0),
                                    stop=(i == PE_TAPS - 1),
                                    skip_group_check=True,
                                )
                        mm.then_inc(pe)

            @block.vector
            def _(vector):
                vector.wait_ge(wsem, 16)
                for k in range(NTILES):
                    if k >= NZ:
                        # z slot free once ACT read tile k-NZ
                        vector.wait_ge(act, k - NZ + 1)
                    for h in range(2):
                        vector.wait_ge(pe, 2 * k + h + 1)
                        h0 = h * HT
                        # z = x3 * w3 + psum  (tap 3 fold)
                        vector.scalar_tensor_tensor(
                            out=zts[k % NZ][:, h0 : h0 + HT],
                            in0=xts[k % NB][:, 3 + h0 : 3 + h0 + HT],
                            scalar=w3col(k),
                            in1=pss[k % 2][:, h0 : h0 + HT],
                            op0=ALU.mult,
                            op1=ALU.add,
                        ).then_inc(dvez)

            @block.scalar
            def _(scalar):
                scalar.dma_start(
                    out=wdg[:, : K * 128], in_=wd_d[:, : K * 128]
                ).then_inc(dsem0, 16)
                scalar.dma_start(out=wt[:, :], in_=w_d[:, :]).then_inc(wsem, 16)
                scalar.dma_start(
                    out=wdg[:, K * 128 :], in_=wd_d[:, K * 128 :]
                ).then_inc(dsem1, 16)
                func = getattr(AF, _ACT_FUNC)
                n_act = 0
                for k in range(NTILES):
                    r0, r1 = x_rows(k)
                    t0 = (k % NT) * T
                    if k >= NB:
                        # yt slot's previous store (tile k-NB) must be done
                        scalar.wait_ge(dout[k % NB], 16 * (k // NB))
                    # last tile: half-granularity to shorten the drain chain
                    halves = (0, 1) if k == NTILES - 1 else (None,)
                    for h in halves:
                        sl = (
                            slice(0, T)
                            if h is None
                            else slice(h * HT, h * HT + HT)
                        )
                        scalar.wait_ge(dvez, 2 * k + 2 if h is None else
                                       2 * k + h + 1)
                        scalar.activation(
                            out=yts[k % NB][:, sl], in_=zts[k % NZ][:, sl],
                            func=func,
                            bias=0.0 if func == AF.Copy else zb,
                            scale=1.0,
                        ).then_inc(act)
                        n_act += 1
                        # the DMA trigger races ahead of the still-streaming
                        # activation write; self-wait on its completion inc
                        scalar.wait_ge(act, n_act)
                        scalar.dma_start(
                            out=y_d[r0:r1, t0 + sl.start : t0 + sl.stop],
                            in_=yts[k % NB][:, sl],
                        ).then_inc(dout[k % NB], 16)
                for i in range(NB):
                    n_stores = len([k for k in range(NTILES) if k % NB == i])
                    if (NTILES - 1) % NB == i:
                        n_stores += 1  # last tile stores in two halves
                    scalar.wait_ge(dout[i], 16 * n_stores)

    return nc


def kernel(x, weight):
    global _last_results
    import ml_dtypes
    from concourse.bass_utils import run_bass_kernel_spmd

    bf16 = ml_dtypes.bfloat16
    x = np.asarray(x, dtype=np.float32)
    weight = np.asarray(weight, dtype=np.float32)

    nc = _build_program()

    in_maps = []
    for core in range(N_CORES):
        sl = slice(core * HC, (core + 1) * HC)
        # [B, S, HC] -> [B, HC, S] -> [ROWS, S] with 3 leading zero columns
        # (the causal padding), row r = b*HC + c
        xs = np.zeros((ROWS, S + 3), bf16)
        xs[:, 3:] = x[:, :, sl].transpose(0, 2, 1).reshape(ROWS, S).astype(bf16)
        ws = weight[sl, :]  # (HC, K)
        w_host = np.zeros((128, NU * K + 1), np.float32)
        wd_host = np.zeros((128, NU * K * 128), bf16)
        idx = np.arange(128)
        for u in range(NU):
            blk = u % (HC // 128)
            wu = ws[blk * 128 : (blk + 1) * 128, :]  # (128, K)
            w_host[:, u * K : (u + 1) * K] = wu
            for i in range(K):
                wd_host[idx, (u * K + i) * 128 + idx] = wu[:, i].astype(bf16)
        in_maps.append({"x": xs, "w": w_host, "wd": wd_host})

    res = run_bass_kernel_spmd(nc, in_maps, list(range(N_CORES)))
    _last_results = res

    out = np.empty((B, S, H), np.float32)
    for core in range(N_CORES):
        sl = slice(core * HC, (core + 1) * HC)
        yc = res.results[core]["y"].astype(np.float32).reshape(B, HC, S)
        out[:, :, sl] = yc.transpose(0, 2, 1)
    return out


# revision 8
# speedup vs baseline: 1.6863x; 1.0356x over previous
"""Causal depthwise conv1d (K=4) + SiLU, sharded over 8 NeuronCores.

Full shapes: x [4, 8192, 2048] f32, weight [2048, 4] f32 -> y [4, 8192, 2048] f32.

Strategy: tensor-parallel over the hidden/channel dim (fully channel
independent, no halo exchange). Each core gets 256 channels, reorganized
host-side to channel-major [B*256, 3+S] (3 leading zero columns provide the
causal padding) so the conv runs along the free dim with channels on SBUF
partitions.

The kernel is HBM-bandwidth bound, so all HBM traffic is bf16: x is converted
host-side (RNE via ml_dtypes), y is written bf16 and upconverted host-side.
That halves traffic vs fp32 (rel-err budget 2e-2 >> bf16's ~2e-3).

Compute split so every engine stays under the ~2.9us/tile bf16 DMA budget:
taps 0-2 run on the TensorEngine as bf16 diagonal-matrix matmuls accumulating
in PSUM (the 32 [128,128] diag matrices are built host-side and DMA'd once);
tap 3 is folded by the DVE scalar_tensor_tensor (z = x3*w3 + psum -> sbuf
bf16); ACT applies SiLU (z -> bf16 out) and triggers the output DMA on its
own HWDGE ring, with inputs streaming on SP's ring.

Ramp shaping (the steady state already runs at the HBM roofline): the diag
DMA is split so unit 0's block lands first and tile 0's input lands in two
halves, pulling the first matmul ~5us earlier; PE/DVE work half-tiles
(per-half sem incs) and the last tile's SiLU + store are split in half, so
the end-of-kernel dependency chain drains ~6us faster.

ACT (silu + store trigger + sem waits) was measured as the steady-state
pacer at ~2.9us/tile, just above the 2.86us DMA floor, and it accumulated a
2-tile backlog by kernel end. So tiles are silued and stored in PAIRS
(adjacent tiles are contiguous in y_d): one 4096-wide ACTIVATE (saves the
352-cycle per-op ramp) and one 1MiB store trigger per two tiles puts ACT at
~2.3us/tile. The last pair keeps the v3 tail exemption (tile 30 single,
tile 31 in halves).

Raw bass (no Tile framework): the installed walrus codegen only accepts one
sync wait per compute instruction, so all synchronization is explicit wait_ge
sequencer instructions. Per-buffer-slot DMA semaphores keep concurrent DMA
completion increments unambiguous. Sem increments fire at instruction
completion, but the sequencer runs ahead, so consumers of an engine's result
always gate on that completion increment (including same-engine self-waits
before DMA triggers).
"""

import contextlib

import numpy as np

B, S, H, K = 4, 8192, 2048, 4
N_CORES = 8
HC = H // N_CORES          # 256 channels per core
ROWS = B * HC              # 1024 rows per core, row r = b*HC + c
NU = ROWS // 128           # 8 partition units
T = 2048                   # token tile
NT = S // T
NTILES = NU * NT           # 32
NB = 6                     # buffers per tile kind
NZ = 3                     # z (pre-silu) buffers
NC_CHUNK = 512             # one PSUM bank of fp32
NCHUNKS = T // NC_CHUNK
PE_TAPS = 3                # taps on the TensorEngine; tap 3 folds in on DVE
HT = T // 2                # half-tile (sem granularity for PE/DVE)
X0SPLIT = HT + 4           # tile-0 first-half DMA columns (covers chunks 0-1)

_last_results = None       # test harness introspection (exec_time_ns etc.)
_ACT_FUNC = "Silu"         # sim override hook (CoreSim lacks Silu)


def _build_program():
    from concourse import bass, mybir

    f32 = mybir.dt.float32
    bf16 = mybir.dt.bfloat16
    AF = mybir.ActivationFunctionType
    ALU = mybir.AluOpType

    nc = bass.Bass()
    # x arrives with 3 leading zero columns (causal padding): [ROWS, 3+S]
    x_d = nc.declare_dram_parameter("x", [ROWS, S + 3], bf16, isOutput=False)
    # 32 host-built [128,128] diag(w) blocks, unit-major then tap
    wd_d = nc.declare_dram_parameter("wd", [128, NU * K * 128], bf16,
                                     isOutput=False)
    # raw weights (fp32): stt scalar columns + a zeros column (Silu bias)
    w_d = nc.declare_dram_parameter("w", [128, NU * K + 1], f32, isOutput=False)
    y_d = nc.declare_dram_parameter("y", [ROWS, S], bf16, isOutput=True)

    with contextlib.ExitStack() as st:
        wt = st.enter_context(nc.sbuf_tensor("wt", [128, NU * K + 1], f32))
        wdg = st.enter_context(nc.sbuf_tensor("wdg", [128, NU * K * 128], bf16))
        xts = [
            st.enter_context(nc.sbuf_tensor(f"xt{i}", [128, T + 3], bf16))
            for i in range(NB)
        ]
        # pair-sized (2 tiles wide) pre-silu and post-silu buffers
        zps = [
            st.enter_context(nc.sbuf_tensor(f"zp{i}", [128, 2 * T], bf16))
            for i in range(NZ)
        ]
        ytp = [
            st.enter_context(nc.sbuf_tensor(f"yp{i}", [128, 2 * T], bf16))
            for i in range(NZ)
        ]
        pss = [
            st.enter_context(nc.psum_tensor(f"ps{i}", [128, T], f32))
            for i in range(2)
        ]
        zb = wt[:, NU * K : NU * K + 1]           # zeros column (Silu bias)

        def wdiag(k, i):
            u = k // NT
            c0 = (u * K + i) * 128
            return wdg[:, c0 : c0 + 128]

        def w3col(k):
            u = k // NT
            return wt[:, u * K + 3 : u * K + 4]

        def x_rows(k):
            r0 = (k // NT) * 128
            return r0, r0 + 128

        # cumulative din counts: tile 0 arrives as two half DMAs
        din_need = []          # din count PE needs before tile k (full tile)
        din_tot = [0] * NB
        for k in range(NTILES):
            din_tot[k % NB] += 32 if k == 0 else 16
            din_need.append(din_tot[k % NB])

        with (
            nc.Block() as block,
            nc.semaphore("wsem") as wsem,
            nc.semaphore("dsem0") as dsem0,
            nc.semaphore("dsem1") as dsem1,
            nc.semaphore("dvez") as dvez,
            nc.semaphore("act") as act,
            nc.semaphore("pe") as pe,
            contextlib.ExitStack() as sems,
        ):
            din = [
                sems.enter_context(nc.semaphore(f"din{i}")) for i in range(NB)
            ]
            dout = [
                sems.enter_context(nc.semaphore(f"dout{i}")) for i in range(NZ)
            ]

            @block.sync
            def _(sync):
                for k in range(NTILES):
                    r0, r1 = x_rows(k)
                    t0 = (k % NT) * T
                    if k >= NB:
                        # xt slot free once DVE folded tap 3 of tile k-NB
                        sync.wait_ge(dvez, 2 * (k - NB) + 2)
                    # padded coords: window [t0-3, t0+T) = x_d cols [t0, t0+T+3)
                    if k == 0:
                        sync.dma_start(
                            out=xts[0][:, :X0SPLIT],
                            in_=x_d[r0:r1, :X0SPLIT],
                        ).then_inc(din[0], 16)
                        sync.dma_start(
                            out=xts[0][:, X0SPLIT : T + 3],
                            in_=x_d[r0:r1, X0SPLIT : T + 3],
                        ).then_inc(din[0], 16)
                    else:
                        sync.dma_start(
                            out=xts[k % NB][:, :],
                            in_=x_d[r0:r1, t0 : t0 + T + 3],
                        ).then_inc(din[k % NB], 16)

            @block.tensor
            def _(tensor):
                tensor.wait_ge(dsem0, 16)
                for k in range(NTILES):
                    if k == 4:
                        tensor.wait_ge(dsem1, 16)
                    if k >= 2:
                        # psum buffer free once DVE consumed tile k-2
                        tensor.wait_ge(dvez, 2 * (k - 2) + 2)
                    ps = pss[k % 2]
                    xt = xts[k % NB]
                    for h in range(2):
                        if k == 0:
                            tensor.wait_ge(din[0], 16 * (h + 1))
                        elif h == 0:
                            tensor.wait_ge(din[k % NB], din_need[k])
                        for c in range(2 * h, 2 * h + 2):
                            c0 = c * NC_CHUNK
                            for i in range(PE_TAPS):
                                mm = tensor.matmul(
                                    ps[:, c0 : c0 + NC_CHUNK],
                                    wdiag(k, i),
                                    xt[:, c0 + i : c0 + i + NC_CHUNK],
                                    start=(i == 0),
                                    stop=(i == PE_TAPS - 1),
                                    skip_group_check=True,
                                )
                        mm.then_inc(pe)

            @block.vector
            def _(vector):
                vector.wait_ge(wsem, 16)
                for k in range(NTILES):
                    j, e = k // 2, k % 2
                    if e == 0 and j >= NZ:
                        # z pair slot free once ACT silued pair j-NZ
                        vector.wait_ge(act, j - NZ + 1)
                    for h in range(2):
                        vector.wait_ge(pe, 2 * k + h + 1)
                        h0 = h * HT
                        # z = x3 * w3 + psum  (tap 3 fold)
                        vector.scalar_tensor_tensor(
                            out=zps[j % NZ][:, e * T + h0 : e * T + h0 + HT],
                            in0=xts[k % NB][:, 3 + h0 : 3 + h0 + HT],
                            scalar=w3col(k),
                            in1=pss[k % 2][:, h0 : h0 + HT],
                            op0=ALU.mult,
                            op1=ALU.add,
                        ).then_inc(dvez)

            @block.scalar
            def _(scalar):
                scalar.dma_start(
                    out=wdg[:, : K * 128], in_=wd_d[:, : K * 128]
                ).then_inc(dsem0, 16)
                scalar.dma_start(out=wt[:, :], in_=w_d[:, :]).then_inc(wsem, 16)
                scalar.dma_start(
                    out=wdg[:, K * 128 :], in_=wd_d[:, K * 128 :]
                ).then_inc(dsem1, 16)
                func = getattr(AF, _ACT_FUNC)
                bias = 0.0 if func == AF.Copy else zb
                NPAIR = NTILES // 2
                n_act = 0
                n_store = [0] * NZ

                def silu_store(j, sl, dvez_need):
                    # silu zps[j%NZ][:, sl] -> ytp, then store that slice
                    nonlocal n_act
                    k0 = 2 * j
                    r0, r1 = x_rows(k0)
                    t0 = (k0 % NT) * T
                    scalar.wait_ge(dvez, dvez_need)
                    scalar.activation(
                        out=ytp[j % NZ][:, sl], in_=zps[j % NZ][:, sl],
                        func=func, bias=bias, scale=1.0,
                    ).then_inc(act)
                    n_act += 1
                    # the DMA trigger races ahead of the still-streaming
                    # activation write; self-wait on its completion inc
                    scalar.wait_ge(act, n_act)
                    scalar.dma_start(
                        out=y_d[r0:r1, t0 + sl.start : t0 + sl.stop],
                        in_=ytp[j % NZ][:, sl],
                    ).then_inc(dout[j % NZ], 16)
                    n_store[j % NZ] += 1

                for j in range(NPAIR):
                    if j >= NZ:
                        # yt pair slot's previous store must be done
                        scalar.wait_ge(dout[j % NZ], 16 * (j // NZ))
                    if j < NPAIR - 1:
                        silu_store(j, slice(0, 2 * T), 4 * j + 4)
                    else:
                        # last pair: finer grain to shorten the drain chain
                        silu_store(j, slice(0, T), 4 * j + 2)
                        silu_store(j, slice(T, T + HT), 4 * j + 3)
                        silu_store(j, slice(T + HT, 2 * T), 4 * j + 4)
                for i in range(NZ):
                    scalar.wait_ge(dout[i], 16 * n_store[i])

    return nc


def kernel(x, weight):
    global _last_results
    import ml_dtypes
    from concourse.bass_utils import run_bass_kernel_spmd

    bf16 = ml_dtypes.bfloat16
    x = np.asarray(x, dtype=np.float32)
    weight = np.asarray(weight, dtype=np.float32)

    nc = _build_program()

    in_maps = []
    for core in range(N_CORES):
        sl = slice(core * HC, (core + 1) * HC)
        # [B, S, HC] -> [B, HC, S] -> [ROWS, S] with 3 leading zero columns
        # (the causal padding), row r = b*HC + c
        xs = np.zeros((ROWS, S + 3), bf16)
        xs[:, 3:] = x[:, :, sl].transpose(0, 2, 1).reshape(ROWS, S).astype(bf16)
        ws = weight[sl, :]  # (HC, K)
        w_host = np.zeros((128, NU * K + 1), np.float32)
        wd_host = np.zeros((128, NU * K * 128), bf16)
        idx = np.arange(128)
        for u in range(NU):
            blk = u % (HC // 128)
            wu = ws[blk * 128 : (blk + 1) * 128, :]  # (128, K)
            w_host[:, u * K : (u + 1) * K] = wu
            for i in range(K):
                wd_host[idx, (u * K + i) * 128 + idx] = wu[:, i].astype(bf16)
        in_maps.append({"x": xs, "w": w_host, "wd": wd_host})

    res = run_bass_kernel_spmd(nc, in_maps, list(range(N_CORES)))
    _last_results = res

    out = np.empty((B, S, H), np.float32)
    for core in range(N_CORES):
        sl = slice(core * HC, (core + 1) * HC)
        yc = res.results[core]["y"].astype(np.float32).reshape(B, HC, S)
        out[:, :, sl] = yc.transpose(0, 2, 1)
    return out
